# revision 8
# baseline (speedup 1.0000x reference)
"""Trainium2 Bass kernel for CartNN minimal-NEAT forward pass.

Computes out = tanh(tanh(x @ w + b))[:, None] for x [16384, 4096] f32,
w [4096] f32, b [1] f32, data-parallel across 8 NeuronCores (2048 batch
rows per core).

Per-core plan (memory-bound; ~32 MiB of x per core):
  - w is DMA-broadcast once to a [128, 4096] SBUF tile, b to [128, 1].
  - x arrives as 16 tiles of [128 partitions, 4096] (contiguous 16 KiB
    per partition -> full DMA line rate).
  - One fused VectorE tensor_tensor_reduce per tile computes
    acc[p] = b + sum_k x[p,k]*w[k]  (mul + reduce-add in a single pass,
    ~2.9 us/tile, well under the ~5.9 us/tile DMA time).
  - Two ScalarE Tanh activations on the [128, 16] accumulator.
  - One 8 KiB DMA writes the [2048, 1] output slice.
"""

import numpy as np

import concourse.bacc as bacc
import concourse.mybir as mybir
from concourse.bass_utils import run_bass_kernel_spmd
from concourse.tile import TileContext

N_CORES = 8
BATCH = 16384
IN_SIZE = 4096
P = 128
B_PER_CORE = BATCH // N_CORES  # 2048
N_TILES = B_PER_CORE // P  # 16

_NC_CACHE = None


def _build():
    nc = bacc.Bacc(
        "TRN2",
        target_bir_lowering=False,
        debug=False,
        num_devices=N_CORES,
    )
    x = nc.dram_tensor(
        "x", [B_PER_CORE, IN_SIZE], mybir.dt.float32, kind="ExternalInput"
    )
    w = nc.dram_tensor("w", [IN_SIZE], mybir.dt.float32, kind="ExternalInput")
    b = nc.dram_tensor("b", [1], mybir.dt.float32, kind="ExternalInput")
    y = nc.dram_tensor("y", [B_PER_CORE, 1], mybir.dt.float32, kind="ExternalOutput")

    xt = x.rearrange("(t p) k -> t p k", p=P)  # [16, 128, 4096]
    yv = y.rearrange("(t p) o -> p (t o)", p=P)  # [128, 16]

    with TileContext(nc) as tc:
        with (
            tc.tile_pool(name="xpool", bufs=4) as xpool,
            tc.tile_pool(name="scratch", bufs=2) as spool,
            tc.tile_pool(name="consts", bufs=1) as cpool,
        ):
            w_PK = cpool.tile([P, IN_SIZE], mybir.dt.float32)
            nc.sync.dma_start(out=w_PK[:], in_=w[None, :].to_broadcast((P, IN_SIZE)))
            b_P1 = cpool.tile([P, 1], mybir.dt.float32)
            nc.sync.dma_start(out=b_P1[:], in_=b[None, :].to_broadcast((P, 1)))
            acc_PT = cpool.tile([P, N_TILES], mybir.dt.float32)

            for t in range(N_TILES):
                x_PK = xpool.tile([P, IN_SIZE], mybir.dt.float32)
                nc.sync.dma_start(out=x_PK[:], in_=xt[t])
                # Fused dot product on VectorE: prod = (x*1 + 0)*w with a
                # free-axis reduce into acc[:, t]. One DVE pass per tile.
                prod_PK = spool.tile([P, IN_SIZE], mybir.dt.float32)
                nc.vector.affine_mul_reduce(
                    out=prod_PK[:],
                    accum_out=acc_PT[:, t : t + 1],
                    in0=x_PK[:],
                    in1=w_PK[:],
                    scale=1.0,
                    bias=0.0,
                )

            h_PT = cpool.tile([P, N_TILES], mybir.dt.float32)
            nc.scalar.activation(
                h_PT[:],
                acc_PT[:],
                mybir.ActivationFunctionType.Tanh,
                bias=b_P1[:],
            )
            nc.scalar.activation(h_PT[:], h_PT[:], mybir.ActivationFunctionType.Tanh)
            nc.sync.dma_start(out=yv, in_=h_PT[:])
    nc.compile()
    return nc


def _get_nc():
    global _NC_CACHE
    if _NC_CACHE is None:
        _NC_CACHE = _build()
    return _NC_CACHE


def _run(x, w, b, **spmd_kwargs):
    """Shard, execute on 8 cores, gather. Returns (out, BassKernelResults)."""
    x = np.ascontiguousarray(np.asarray(x, dtype=np.float32))
    w = np.ascontiguousarray(np.asarray(w, dtype=np.float32))
    b = np.ascontiguousarray(np.asarray(b, dtype=np.float32))
    assert x.shape == (BATCH, IN_SIZE), x.shape

    nc = _get_nc()
    in_maps = [
        {"x": x[c * B_PER_CORE : (c + 1) * B_PER_CORE], "w": w, "b": b}
        for c in range(N_CORES)
    ]
    res = run_bass_kernel_spmd(nc, in_maps, list(range(N_CORES)), **spmd_kwargs)
    out = np.concatenate(
        [np.asarray(res.results[c]["y"]) for c in range(N_CORES)], axis=0
    )
    return out.astype(np.float32, copy=False), res


def kernel(x, w, b):
    out, _ = _run(x, w, b)
    return out
